# revision 11
# baseline (speedup 1.0000x reference)
"""Trainium2 Bass kernel for an attention-decoder LSTM (nn_Decoder).

Data-parallel over 8 NeuronCores: batch 4096 -> 512 per core. All weights
replicated. The T-1=127 step recurrence runs fully on-chip: enc_proj is
precomputed once into SBUF (bf16, [ENC, T, B] layout) and every step does
  hp   = 0.5*W1_h.T @ H + 0.5*W1_c.T @ C          (PE, H=2h, C=2c)
  X    = tanh(enc_proj + hp)                       (DVE add + ACT tanh->fp8)
  e    = w2.T @ X via fp8 DoubleRow matmuls        (PE, 2 timesteps/matmul)
  S    = exp(e/SW)                                 (ACT)
  den  = ones.T @ S ; num = ones.T @ (S*pfc)       (PE)
  r    = num / den                                 (DVE reciprocal + mult)
  gates= whh.T @ H + [wih;bias].T @ [yt;1]         (PE; per-gate scale folded)
  LSTM update via tanh-only form (single fused gate tanh on ACT)
Final output row: 0.5*Wfh.T @ H + (ones.T @ (S*pfin))/den + fc_final_b.
"""

import numpy as np
import ml_dtypes

import concourse.bass as bass
import concourse.bacc as bacc
import concourse.tile as tile
from concourse import mybir
from concourse.bass_utils import run_bass_kernel_spmd

NCORES = 8
B_FULL, T, E, D = 4096, 128, 128, 128
B = B_FULL // NCORES        # 512 batch per core
TSTEPS = T - 1              # 127
TC = 8                      # t-chunk for the big add/tanh passes
NBLK = B // 128             # 4 b-blocks of 128 for input transpose
SW = 16.0                   # fp8 scale on w2 (undone in the exp)
FP8_EDOT = True             # e-dot via fp8 DoubleRow (else bf16 one-hot)
MICROBENCH = True

FP = mybir.dt.float32
BF = mybir.dt.bfloat16
F8 = mybir.dt.float8e4
AF = mybir.ActivationFunctionType
OP = mybir.AluOpType
PM = mybir.MatmulPerfMode
BF_NP = ml_dtypes.bfloat16
F8_NP = ml_dtypes.float8_e4m3


def _build(fc_wy: float, fc_final_b: float, n_steps: int):
    nc = bacc.Bacc("TRN2", target_bir_lowering=False, debug=False,
                   num_devices=NCORES)

    x_ext = nc.declare_dram_parameter("x", [B, T, E], FP, isOutput=False)
    yh_ext = nc.declare_dram_parameter("yh", [TSTEPS, B], BF, isOutput=False)
    # [0.5*W1_h.T | 0.5*W1_c.T]  -> [D, 2E]
    w1hc_ext = nc.declare_dram_parameter("w1hc", [D, 2 * E], BF, isOutput=False)
    wke_ext = nc.declare_dram_parameter("wke", [E, E], BF, isOutput=False)  # W1_e.T
    # fp8 DoubleRow one-hot stationary: [:,0,T-1]=w2*SW, [:,1,T]=w2*SW
    w2g2_ext = nc.declare_dram_parameter("w2g2", [E, 2, 2 * T], F8,
                                         isOutput=False)
    w2g_ext = nc.declare_dram_parameter("w2g", [E, 2 * T], BF, isOutput=False)
    gfc_ext = nc.declare_dram_parameter("gfc", [E, 2 * T], BF, isOutput=False)
    gfin_ext = nc.declare_dram_parameter("gfin", [E, 2 * T], BF, isOutput=False)
    # per-gate scale folded: s_g*0.5*W_hh.T
    whh_ext = nc.declare_dram_parameter("whh", [D, 4 * D], BF, isOutput=False)
    # rank-2 gate tail: row0 = s_g*W_ih col, row1 = s_g*gate bias
    wb2_ext = nc.declare_dram_parameter("wb2", [2, 4 * D], BF, isOutput=False)
    b1_ext = nc.declare_dram_parameter("b1", [E, 1], FP, isOutput=False)
    wfh_ext = nc.declare_dram_parameter("wfh", [D, 1], BF, isOutput=False)  # 0.5*Wfh
    id_ext = nc.declare_dram_parameter("ident", [128, 128], BF, isOutput=False)
    out_ext = nc.declare_dram_parameter("out", [1, B], FP, isOutput=True)

    with tile.TileContext(nc) as tc:
        import contextlib
        _stack = contextlib.ExitStack()
        const = _stack.enter_context(tc.tile_pool(name="const", bufs=1))
        dma4 = _stack.enter_context(tc.tile_pool(name="dma4", bufs=4))

        # ---- constants -------------------------------------------------
        w1hc_sb = const.tile([D, 2 * E], BF, tag="w1hc")
        nc.sync.dma_start(out=w1hc_sb[:], in_=w1hc_ext[:])
        wke_sb = const.tile([E, E], BF, tag="wke")
        nc.sync.dma_start(out=wke_sb[:], in_=wke_ext[:])
        if FP8_EDOT:
            w2g2_sb = const.tile([E, 2, 2 * T], F8, tag="w2g2")
            nc.sync.dma_start(out=w2g2_sb[:], in_=w2g2_ext[:])
        else:
            w2g_sb = const.tile([E, 2 * T], BF, tag="w2g")
            nc.sync.dma_start(out=w2g_sb[:], in_=w2g_ext[:])
        gfc_sb = const.tile([E, 2 * T], BF, tag="gfc")
        nc.sync.dma_start(out=gfc_sb[:], in_=gfc_ext[:])
        gfin_sb = const.tile([E, 2 * T], BF, tag="gfin")
        nc.sync.dma_start(out=gfin_sb[:], in_=gfin_ext[:])
        whh_sb = const.tile([D, 4 * D], BF, tag="whh")
        nc.sync.dma_start(out=whh_sb[:], in_=whh_ext[:])
        wb2_sb = const.tile([2, 4 * D], BF, tag="wb2")
        nc.sync.dma_start(out=wb2_sb[:], in_=wb2_ext[:])
        b1_sb = const.tile([E, 1], FP, tag="b1")
        nc.sync.dma_start(out=b1_sb[:], in_=b1_ext[:])
        wfh_sb = const.tile([D, 1], BF, tag="wfh")
        nc.sync.dma_start(out=wfh_sb[:], in_=wfh_ext[:])
        id_sb = const.tile([128, 128], BF, tag="ident")
        nc.sync.dma_start(out=id_sb[:], in_=id_ext[:])
        ones_sb = const.tile([T, 1], BF, tag="ones")
        nc.vector.memset(ones_sb[:], 1.0)
        ytones = const.tile([2, B], BF, tag="ytones")
        nc.vector.memset(ytones[:], 1.0)   # row0 overwritten per step

        encp = const.tile([E, T, B], BF, tag="encp")
        pfc_sb = const.tile([T, B], BF, tag="pfc")
        pfin_sb = const.tile([T, B], BF, tag="pfin")
        H = const.tile([D, B], FP, tag="H")   # 2*h
        C = const.tile([D, B], FP, tag="C")   # 2*c
        nc.vector.memset(H[:], 0.0)
        nc.vector.memset(C[:], 0.0)

        if MICROBENCH:
            mbA = const.tile([128, 1600], BF, tag="mbA")
            mbB = const.tile([128, 1600], BF, tag="mbB")
            nc.vector.memset(mbA[:], 0.7)
            nc.vector.memset(mbB[:], 1.5)

        # ---- precompute: enc_proj, pfc, pfin ---------------------------
        with contextlib.ExitStack() as pre:
            pwork = pre.enter_context(tc.tile_pool(name="pwork", bufs=2))
            pps = pre.enter_context(tc.tile_pool(name="pps", bufs=4,
                                                 space="PSUM"))
            pps2 = pre.enter_context(tc.tile_pool(name="pps2", bufs=2,
                                                  space="PSUM"))
            pfc_ps = pps2.tile([T, B], FP, tag="p2")
            pfin_ps = pps2.tile([T, B], FP, tag="p2")
            for t in range(T):
                inT_ps = pps.tile([E, B], BF, tag="big")
                for blk in range(NBLK):
                    xin = dma4.tile([128, E], FP, tag="xin")
                    nc.sync.dma_start(
                        out=xin[:],
                        in_=x_ext[blk * 128:(blk + 1) * 128, t, :],
                    )
                    xbf = pwork.tile([128, E], BF, tag="xbf")
                    nc.vector.tensor_copy(xbf[:], xin[:])
                    nc.tensor.transpose(
                        inT_ps[:, blk * 128:(blk + 1) * 128], xbf[:], id_sb[:]
                    )
                inT = pwork.tile([E, B], BF, tag="inT")
                nc.vector.tensor_copy(inT[:], inT_ps[:])
                ep_ps = pps.tile([E, B], FP, tag="big")
                nc.tensor.matmul(ep_ps[:], wke_sb[:], inT[:],
                                 start=True, stop=True)
                nc.tensor.matmul(pfc_ps[:], gfc_sb[:, T - 1 - t:2 * T - 1 - t],
                                 inT[:], start=(t == 0), stop=(t == T - 1))
                nc.tensor.matmul(pfin_ps[:], gfin_sb[:, T - 1 - t:2 * T - 1 - t],
                                 inT[:], start=(t == 0), stop=(t == T - 1))
                # enc_proj + attn_b1, cast to bf16, store [E, t, B]
                nc.scalar.activation(encp[:, t, :], ep_ps[:],
                                     AF.Identity, bias=b1_sb[:], scale=1.0)
            nc.vector.tensor_copy(pfc_sb[:], pfc_ps[:])
            nc.vector.tensor_copy(pfin_sb[:], pfin_ps[:])

        # ---- main pools -----------------------------------------------
        xb = _stack.enter_context(tc.tile_pool(name="xb", bufs=2))
        xf = _stack.enter_context(tc.tile_pool(name="xf", bufs=2))
        wk = _stack.enter_context(tc.tile_pool(name="wk", bufs=2))
        wk1 = _stack.enter_context(tc.tile_pool(name="wk1", bufs=1))
        ps_e = _stack.enter_context(tc.tile_pool(name="ps_e", bufs=1,
                                                 space="PSUM"))
        ps_hp = _stack.enter_context(tc.tile_pool(name="ps_hp", bufs=1,
                                                  space="PSUM"))
        ps_g = _stack.enter_context(tc.tile_pool(name="ps_g", bufs=1,
                                                 space="PSUM"))
        ps_dn = _stack.enter_context(tc.tile_pool(name="ps_dn", bufs=2,
                                                  space="PSUM"))

        # initial bf16 state casts (zeros)
        Hbf = wk.tile([D, B], BF, tag="Hbf")
        Cbf = wk.tile([D, B], BF, tag="Cbf")
        nc.vector.memset(Hbf[:], 0.0)
        nc.vector.memset(Cbf[:], 0.0)

        rcp = None
        S_sb = None

        # ---- the recurrence -------------------------------------------
        for s in range(n_steps):
            yrow = dma4.tile([1, B], BF, tag="yrow")
            nc.sync.dma_start(out=yrow[:], in_=yh_ext[s:s + 1, :])
            # hp = 0.5*W1h.T @ H + 0.5*W1c.T @ C   [E, B]
            hp_ps = ps_hp.tile([E, B], FP, tag="hp")
            nc.tensor.matmul(hp_ps[:], w1hc_sb[:, 0:E], Hbf[:],
                             start=True, stop=False)
            nc.tensor.matmul(hp_ps[:], w1hc_sb[:, E:2 * E], Cbf[:],
                             start=False, stop=True)
            hp_sb = wk.tile([E, B], BF, tag="hp_sb")
            nc.vector.tensor_copy(hp_sb[:], hp_ps[:])
            hp_b = hp_sb[:].unsqueeze(1).broadcast_to([E, TC, B])

            e_ps = ps_e.tile([T, B], FP, tag="e")
            for tcid in range(T // TC):
                Xb = xb.tile([E, TC, B], BF, tag="Xb")
                nc.vector.tensor_tensor(
                    Xb[:], encp[:, tcid * TC:(tcid + 1) * TC, :], hp_b,
                    op=OP.add)
                if FP8_EDOT:
                    Xf = xf.tile([E, TC, B], F8, tag="Xf")
                    nc.scalar.activation(Xf[:], Xb[:], AF.Tanh)
                    for j in range(0, TC, 2):
                        t = tcid * TC + j
                        nc.tensor.matmul(
                            e_ps[:], w2g2_sb[:, :, T - 1 - t:2 * T - 1 - t],
                            Xf[:, j:j + 2, :],
                            start=(t == 0), stop=(t == T - 2),
                            perf_mode=PM.DoubleRow)
                else:
                    nc.scalar.activation(Xb[:], Xb[:], AF.Tanh)
                    for j in range(TC):
                        t = tcid * TC + j
                        nc.tensor.matmul(
                            e_ps[:], w2g_sb[:, T - 1 - t:2 * T - 1 - t],
                            Xb[:, j, :], start=(t == 0), stop=(t == T - 1))

            S_sb = wk.tile([T, B], BF, tag="S")
            nc.scalar.activation(S_sb[:], e_ps[:], AF.Exp,
                                 scale=(1.0 / SW) if FP8_EDOT else 1.0)
            SP = wk.tile([T, B], BF, tag="SP")
            nc.vector.tensor_tensor(SP[:], S_sb[:], pfc_sb[:], op=OP.mult)

            den_ps = ps_dn.tile([1, B], FP, tag="p2")
            nc.tensor.matmul(den_ps[:], ones_sb[:], S_sb[:],
                             start=True, stop=True)
            num_ps = ps_dn.tile([1, B], FP, tag="p2")
            nc.tensor.matmul(num_ps[:], ones_sb[:], SP[:],
                             start=True, stop=True)

            rcp = wk1.tile([1, B], FP, tag="rcp")
            nc.vector.reciprocal(rcp[:], den_ps[:])
            r = wk1.tile([1, B], FP, tag="r")
            nc.vector.tensor_tensor(r[:], num_ps[:], rcp[:], op=OP.mult)
            # y_tilde (sans fc_b, folded into gate bias) as bf16 row,
            # written straight into the rank-2 moving tile [yt; ones]
            nc.vector.scalar_tensor_tensor(ytones[0:1, :], yrow[:],
                                           fc_wy, r[:],
                                           op0=OP.mult, op1=OP.add)

            # gates: g = s_g*(0.5*Whh.T @ H) + [s_g*wih; s_g*bias].T @ [yt; 1]
            g_ps = ps_g.tile([D, 4, B], FP, tag="g")
            for g in range(4):
                nc.tensor.matmul(g_ps[:, g, :], whh_sb[:, g * D:(g + 1) * D],
                                 Hbf[:], start=True, stop=False)
                nc.tensor.matmul(g_ps[:, g, :], wb2_sb[:, g * D:(g + 1) * D],
                                 ytones[:], start=False, stop=True)
            tg = wk1.tile([D, 4, B], FP, tag="tg")
            nc.scalar.activation(tg[:], g_ps[:], AF.Tanh)

            # C_new(=2c) = 0.5*(tf+1)*C + (ti+1)*tg ; H_new(=2h) = (to+1)*tanh(c)
            tmp1 = wk1.tile([D, B], FP, tag="tmp1")
            nc.vector.scalar_tensor_tensor(tmp1[:], tg[:, 1, :], 1.0, C[:],
                                           op0=OP.add, op1=OP.mult)
            tmp2 = wk1.tile([D, B], FP, tag="tmp2")
            nc.vector.scalar_tensor_tensor(tmp2[:], tg[:, 0, :], 1.0,
                                           tg[:, 2, :],
                                           op0=OP.add, op1=OP.mult)
            nc.vector.scalar_tensor_tensor(C[:], tmp1[:], 0.5, tmp2[:],
                                           op0=OP.mult, op1=OP.add)
            tct = wk1.tile([D, B], FP, tag="tct")
            nc.scalar.activation(tct[:], C[:], AF.Tanh, scale=0.5)
            nc.vector.scalar_tensor_tensor(H[:], tg[:, 3, :], 1.0, tct[:],
                                           op0=OP.add, op1=OP.mult)
            Hbf = wk.tile([D, B], BF, tag="Hbf")
            nc.vector.tensor_copy(Hbf[:], H[:])
            Cbf = wk.tile([D, B], BF, tag="Cbf")
            nc.vector.tensor_copy(Cbf[:], C[:])

        # ---- final output row ----------------------------------------
        o_ps = ps_dn.tile([1, B], FP, tag="p2")
        nc.tensor.matmul(o_ps[:], wfh_sb[:], Hbf[:], start=True, stop=True)
        if n_steps > 0:
            SPf = wk.tile([T, B], BF, tag="SP")
            nc.vector.tensor_tensor(SPf[:], S_sb[:], pfin_sb[:], op=OP.mult)
            nf_ps = ps_dn.tile([1, B], FP, tag="p2")
            nc.tensor.matmul(nf_ps[:], ones_sb[:], SPf[:], start=True, stop=True)
            rfin = wk1.tile([1, B], FP, tag="r")
            nc.vector.tensor_tensor(rfin[:], nf_ps[:], rcp[:], op=OP.mult)
            o_sb = wk1.tile([1, B], FP, tag="osb")
            nc.vector.scalar_tensor_tensor(o_sb[:], o_ps[:], fc_final_b, rfin[:],
                                           op0=OP.add, op1=OP.add)
        else:
            o_sb = wk1.tile([1, B], FP, tag="osb")
            nc.vector.tensor_scalar_add(o_sb[:], o_ps[:], fc_final_b)
        nc.sync.dma_start(out=out_ext[:], in_=o_sb[:])

        if MICROBENCH:
            # timing probes, read from the trace by free-dim size:
            #   TT divide @2400, gpsimd add @1200, STT @2000
            with nc.allow_low_precision(reason="timing probe"):
                nc.vector.reciprocal(mbA[:], mbB[:])
            nc.gpsimd.tensor_tensor(mbA[:, 0:800], mbA[:, 0:800],
                                    mbB[:, 0:800], op=OP.add)
            nc.vector.scalar_tensor_tensor(mbA[:, 0:1200], mbA[:, 0:1200], 1.0,
                                           mbB[:, 0:1200],
                                           op0=OP.add, op1=OP.mult)
        _stack.close()

    nc.finalize()
    return nc


def _prep_host(inputs, n_steps):
    f32 = np.float32
    attn_W1 = np.asarray(inputs["attn_W1"], f32)
    attn_W2 = np.asarray(inputs["attn_W2"], f32)
    W_ih = np.asarray(inputs["W_ih"], f32)
    W_hh = np.asarray(inputs["W_hh"], f32)
    b_ih = np.asarray(inputs["b_ih"], f32)
    b_hh = np.asarray(inputs["b_hh"], f32)
    fc_W = np.asarray(inputs["fc_W"], f32)
    fc_b = np.asarray(inputs["fc_b"], f32)
    fcf_W = np.asarray(inputs["fc_final_W"], f32)
    fcf_b = np.asarray(inputs["fc_final_b"], f32)

    W1_h = attn_W1[:, :D]
    W1_c = attn_W1[:, D:2 * D]
    W1_e = attn_W1[:, 2 * D:]

    w1hc = np.concatenate([0.5 * W1_h.T, 0.5 * W1_c.T], axis=1)      # [D, 2E]
    wke = np.ascontiguousarray(W1_e.T)                                # [E, E]
    def onehot_shift(vec):
        g = np.zeros((E, 2 * T), f32)
        g[:, T - 1] = vec
        return g.astype(BF_NP)
    w2g = onehot_shift(attn_W2[0])
    w2g2 = np.zeros((E, 2, 2 * T), f32)
    w2g2[:, 0, T - 1] = attn_W2[0] * SW
    w2g2[:, 1, T] = attn_W2[0] * SW
    gfc = onehot_shift(fc_W[0, :E])
    gfin = onehot_shift(fcf_W[0, D:])
    # per-gate tanh input scale (tanh-only LSTM form), folded into weights
    scales = np.array([0.5, 0.5, 1.0, 0.5], f32)
    sg = np.repeat(scales, D)                                         # [4D]
    whh = 0.5 * W_hh.T * sg[None, :]                                  # [D, 4D]
    wih_row = W_ih[:, 0] * sg                                         # [4D]
    bias_row = (b_ih + b_hh + W_ih[:, 0] * float(fc_b[0])) * sg       # [4D]
    wb2 = np.stack([wih_row, bias_row], axis=0)                       # [2, 4D]
    fc_wy = float(fc_W[0, E])
    wfh = 0.5 * fcf_W[0, :D][:, None]                                 # [D, 1]
    b1 = np.asarray(inputs["attn_b1"], f32)[:, None]

    weights = {
        "w1hc": w1hc.astype(BF_NP), "wke": wke.astype(BF_NP),
        "w2g": w2g, "w2g2": w2g2.astype(F8_NP),
        "gfc": gfc, "gfin": gfin, "whh": whh.astype(BF_NP),
        "wb2": wb2.astype(BF_NP),
        "b1": b1.astype(f32),
        "wfh": wfh.astype(BF_NP),
        "ident": np.eye(128, dtype=f32).astype(BF_NP),
    }

    x_full = np.ascontiguousarray(np.asarray(inputs["input_encoded"], f32))
    yh_full = np.asarray(inputs["y_history"], f32)[:, :, 0]           # [B_FULL, 127]

    in_maps = []
    for i in range(NCORES):
        sl = slice(i * B, (i + 1) * B)
        m = dict(weights)
        m["x"] = x_full[sl]
        m["yh"] = np.ascontiguousarray(yh_full[sl].T).astype(BF_NP)   # [127, B]
        in_maps.append(m)
    return in_maps, fc_wy, float(fcf_b[0])


_RUN_KW = {}


def _kernel_impl(inputs, n_steps):
    in_maps, fc_wy, fcf_b = _prep_host(inputs, n_steps)
    nc = _build(fc_wy, fcf_b, n_steps)
    res = run_bass_kernel_spmd(nc, in_maps, core_ids=list(range(NCORES)),
                               **_RUN_KW)
    out = np.concatenate(
        [np.asarray(res.results[i]["out"], np.float32).reshape(B, 1)
         for i in range(NCORES)], axis=0)
    return out, res


def kernel(**inputs) -> np.ndarray:
    out, _ = _kernel_impl(inputs, TSTEPS)
    return out
